# revision 7
# baseline (speedup 1.0000x reference)
"""CausalBiTrilinearBCNAttention Trainium2 kernel (layout-B rewrite).

Math: the network collapses to xp = x @ P (448 rank columns), causal
cumsums over 4 of the 7 rank groups, elementwise rank products, and a
final [T,128]@[128,D] projection (see P/A folding below).

This version keeps FEATURES on partitions and TOKENS on the free dim:

  xpT = P.T @ x.T        4 stationary groups of <=128 P-columns,
                         tokens stream as the moving operand
  cums = tensor_tensor_scan (DVE prefix-add along free dim, fp32 state)
  ew   = lane-aligned DVE products (P column order is chosen so every
         product pairs values living on the same partitions):
           P cols = [b3|b1 | b7|b2 | a3|a1 | a2s]
           C0 = scan(G3) = [c3@lo | c1@hi]   C0p = C0 * invc
           C1 = scan(G4) = [c7@lo | c2@hi]   C1p = C1 * invc
           g2 = a3 * (c3'*c7')  @lo          -> GT[0:64]
           g1 = a1*c1' + a2s*c2'  @hi        -> GT[64:128]
  out  = A'.T @ GT       A' = [alpha_tri*WO@U_t | WO@U_b]

so there are no PE transposes, no PE cumsum/carry chain, and the PE
stream is 64 xpT MMs + 16 final MMs, all N=512, back to back (HAM
stays warm; a dummy-MM burst warms it during the DMA lead-in).

Sharding: 8 cores = 4 batches x 2 T-halves. The T/2 carry for the
second half is folded by the host into the scan initial values
(carry = sum_t x[b,:T/2] @ P, fp32) and chained across the two
512-token device halves via the scan output's last column.
"""

import numpy as np

import concourse.bass as bass
import concourse.tile as tile
from concourse import bacc, mybir
from concourse.bass_utils import run_bass_kernel_spmd

B, T, D, R = 4, 2048, 1024, 64
TH = T // 2          # tokens per core
ND = D // 128        # 8 contraction chunks
PCOLS = 448
HW = 512             # tokens per device half
F32 = mybir.dt.float32
F16 = mybir.dt.float16

# P' column groups (each <=128 wide -> one stationary tile):
#   G3  = cols   0:128 = [b3|b1]   (cumsummed)
#   G4  = cols 128:256 = [b7|b2]   (cumsummed)
#   A1  = cols 256:384 = [a3|a1]
#   A2s = cols 384:448 = [a2s]     (M=64, written to partitions 64:128)
G3_, G4_, A1_, A2s_ = (0, 128), (128, 256), (256, 384), (384, 448)

ADD = mybir.AluOpType.add
BYP = mybir.AluOpType.bypass
MUL = mybir.AluOpType.mult


def build_nc():
    nc = bacc.Bacc(None, target_bir_lowering=False)

    # x5: [128, pair, half, chunk-in-pair, tok] -- 2KB contiguous/partition
    x5 = nc.dram_tensor("x5", [128, ND // 2, 2, 2, HW], F16,
                        kind="ExternalInput")
    P5 = nc.dram_tensor("P5", [128, ND, PCOLS], F16, kind="ExternalInput")
    AT = nc.dram_tensor("AT", [128, D], F16, kind="ExternalInput")
    invcr = nc.dram_tensor("invcr", [1, TH], F16, kind="ExternalInput")
    carry = nc.dram_tensor("carry", [128, 2], F32, kind="ExternalInput")
    # out5: [128, half, chunk, tok] -- 2KB contiguous/partition per store
    out5 = nc.dram_tensor("out5", [128, 2, ND, HW], F16,
                          kind="ExternalOutput")

    with tile.TileContext(nc) as tc:
        with tc.tile_pool(name="consts", bufs=1) as consts, \
             tc.tile_pool(name="big", bufs=1) as big, \
             tc.tile_pool(name="outp", bufs=4) as outp, \
             tc.tile_pool(name="ps", bufs=1, space="PSUM") as ps:

            # ---- HAM warmup + on-device invc broadcast ----
            warm_sb = consts.tile([128, 128], F16)
            nc.gpsimd.memset(warm_sb, 0.0)
            ones1 = consts.tile([1, 128], F16)
            nc.gpsimd.memset(ones1, 1.0)
            warm_ps = ps.tile([128, 512], F32, tag="out", bufs=3)
            for _ in range(10):
                nc.tensor.matmul(warm_ps[:, 0:128], warm_sb, warm_sb,
                                 start=True, stop=True)

            xT_sb = big.tile([128, ND, TH], F16)
            P_sb = consts.tile([128, ND, PCOLS], F16)
            AT_sb = consts.tile([128, D], F16)
            invcr_sb = consts.tile([1, TH], F16)
            invcT_sb = consts.tile([128, TH], F16)
            carry_sb = consts.tile([128, 2], F32)

            C0raw = big.tile([128, 2, HW], F16)
            C1raw = big.tile([128, 2, HW], F16)
            C0p = big.tile([128, 2, HW], F16)
            C1p = big.tile([128, 2, HW], F16)
            A1sb = big.tile([128, 2, HW], F16)
            A2sb = big.tile([128, 2, HW], F16)
            t2sb = big.tile([64, 2, HW], F16)
            m1sb = big.tile([128, 2, HW], F16)
            m2sb = big.tile([128, 2, HW], F16)
            GT = big.tile([128, 2, HW], F16)

            # ---- input DMAs (per-partition-contiguous units) ----
            def ldx(q, j, h):
                q.dma_start(out=xT_sb[:, 2 * j:2 * j + 2,
                                      h * HW:(h + 1) * HW],
                            in_=x5[:, j, h, :, :])

            nc.sync.dma_start(out=P_sb[:, 0:4, :], in_=P5[:, 0:4, :])
            ldx(nc.sync, 1, 0)
            ldx(nc.sync, 3, 0)
            ldx(nc.sync, 1, 1)
            ldx(nc.sync, 3, 1)

            nc.scalar.dma_start(out=invcr_sb, in_=invcr[:, :])
            nc.scalar.dma_start(out=carry_sb, in_=carry[:, :])
            ldx(nc.scalar, 0, 0)
            nc.scalar.dma_start(out=P_sb[:, 4:8, :], in_=P5[:, 4:8, :])
            ldx(nc.scalar, 2, 0)
            nc.scalar.dma_start(out=AT_sb, in_=AT[:, :])
            ldx(nc.scalar, 0, 1)
            ldx(nc.scalar, 2, 1)

            # invc broadcast rows: PE K=1 outer product, drain via ScalarE
            for h in range(2):
                ivp = ps.tile([128, HW], F32, tag="invb", bufs=1,
                              name=f"ivp{h}")
                nc.tensor.matmul(ivp, ones1, invcr_sb[0:1,
                                                      h * HW:(h + 1) * HW],
                                 start=True, stop=True)
                nc.scalar.copy(invcT_sb[:, h * HW:(h + 1) * HW], ivp)

            def xh(h, dk):
                return xT_sb[:, dk, h * HW:(h + 1) * HW]

            def mm(gp, cols, h, dk):
                lo, hi = cols
                out_ap = gp[64:128, :] if cols is A2s_ else gp
                nc.tensor.matmul(out_ap, P_sb[:, dk, lo:hi], xh(h, dk),
                                 start=(dk == 0), stop=(dk == ND - 1))

            # ============ half 0: dk-pair-blocked sweep (DMA-paced) ========
            g3p = ps.tile([128, HW], F32, tag="g3", bufs=1)
            g4p = ps.tile([128, HW], F32, tag="g4", bufs=1)
            a1p = ps.tile([128, HW], F32, tag="a1", bufs=1)
            a2p = ps.tile([128, HW], F32, tag="a2s", bufs=1)
            for j in range(ND // 2):
                for gp, cols in ((g3p, G3_), (g4p, G4_), (a1p, A1_),
                                 (a2p, A2s_)):
                    mm(gp, cols, 0, 2 * j)
                    mm(gp, cols, 0, 2 * j + 1)

            # h0 scans (raw cums; data1 is an ignored resident tile)
            dum = xT_sb[:, 0, 0:HW]
            nc.vector.tensor_tensor_scan(C0raw[:, 0, :], g3p, dum,
                                         carry_sb[:, 0:1], ADD, BYP)
            nc.vector.tensor_tensor_scan(C1raw[:, 0, :], g4p, dum,
                                         carry_sb[:, 1:2], ADD, BYP)
            nc.vector.tensor_mul(C0p[:, 0, :], C0raw[:, 0, :],
                                 invcT_sb[:, 0:HW])
            nc.vector.tensor_mul(C1p[:, 0, :], C1raw[:, 0, :],
                                 invcT_sb[:, 0:HW])
            nc.scalar.copy(A1sb[:, 0, :], a1p)
            nc.scalar.copy(A2sb[64:128, 0, :], a2p[64:128, :])
            nc.vector.tensor_mul(m1sb[64:128, 0, :], A1sb[64:128, 0, :],
                                 C0p[64:128, 0, :])
            nc.vector.tensor_mul(m2sb[64:128, 0, :], A2sb[64:128, 0, :],
                                 C1p[64:128, 0, :])
            nc.vector.tensor_add(GT[64:128, 0, :], m1sb[64:128, 0, :],
                                 m2sb[64:128, 0, :])
            nc.gpsimd.tensor_mul(t2sb[:, 0, :], C0p[0:64, 0, :],
                                 C1p[0:64, 0, :])
            nc.vector.tensor_mul(GT[0:64, 0, :], A1sb[0:64, 0, :],
                                 t2sb[:, 0, :])

            # ============ half 1 =========
            g3p1 = ps.tile([128, HW], F32, tag="g3", bufs=1)
            for dk in range(ND):
                mm(g3p1, G3_, 1, dk)
            g4p1 = ps.tile([128, HW], F32, tag="g4", bufs=1)
            for dk in range(ND):
                mm(g4p1, G4_, 1, dk)

            nc.vector.tensor_tensor_scan(C0raw[:, 1, :], g3p1, dum,
                                         C0raw[:, 0, HW - 1:HW], ADD, BYP)
            nc.vector.tensor_tensor_scan(C1raw[:, 1, :], g4p1, dum,
                                         C1raw[:, 0, HW - 1:HW], ADD, BYP)
            nc.vector.tensor_mul(C0p[:, 1, :], C0raw[:, 1, :],
                                 invcT_sb[:, HW:TH])
            nc.vector.tensor_mul(C1p[:, 1, :], C1raw[:, 1, :],
                                 invcT_sb[:, HW:TH])

            a1p1 = ps.tile([128, HW], F32, tag="a1", bufs=1)
            for dk in range(ND):
                mm(a1p1, A1_, 1, dk)
            nc.scalar.copy(A1sb[:, 1, :], a1p1)

            def emit_final(h, vdks):
                for dk in range(ND):
                    o_ps = ps.tile([128, 512], F32, tag="out", bufs=3,
                                   name=f"ops{h}_{dk}")
                    nc.tensor.matmul(o_ps, AT_sb[:, dk * 128:(dk + 1) * 128],
                                     GT[:, h, :], start=True, stop=True)
                    if dk % 2 == 0:
                        osb = outp.tile([128, 2, HW], F16,
                                        name=f"osb{h}_{dk}")
                    if dk in vdks:
                        nc.vector.tensor_copy(osb[:, dk % 2, :], o_ps)
                    else:
                        nc.scalar.copy(osb[:, dk % 2, :], o_ps)
                    if dk % 2 == 1:
                        q = nc.sync if (dk // 2) % 2 == 0 else nc.scalar
                        q.dma_start(out=out5[:, h, dk - 1:dk + 1, :], in_=osb)

            nc.vector.tensor_mul(m1sb[64:128, 1, :], A1sb[64:128, 1, :],
                                 C0p[64:128, 1, :])
            nc.gpsimd.tensor_mul(t2sb[:, 1, :], C0p[0:64, 1, :],
                                 C1p[0:64, 1, :])
            nc.vector.tensor_mul(GT[0:64, 1, :], A1sb[0:64, 1, :],
                                 t2sb[:, 1, :])

            emit_final(0, vdks=(0, 2, 4, 6))

            a2p1 = ps.tile([128, HW], F32, tag="a2s", bufs=1)
            for dk in range(ND):
                mm(a2p1, A2s_, 1, dk)
            nc.scalar.copy(A2sb[64:128, 1, :], a2p1[64:128, :])
            nc.vector.tensor_mul(m2sb[64:128, 1, :], A2sb[64:128, 1, :],
                                 C1p[64:128, 1, :])
            nc.vector.tensor_add(GT[64:128, 1, :], m1sb[64:128, 1, :],
                                 m2sb[64:128, 1, :])

            emit_final(1, vdks=(0, 2, 4, 6))

    nc.finalize()
    return nc


_NC = None


def _get_nc():
    global _NC
    if _NC is None:
        _NC = build_nc()
    return _NC


def _fold_weights(WQ, WK, WO, Winv, U_b, V_b, W_b, U_t, V_t, W_t, X_t,
                  alpha_bi, alpha_tri):
    f8 = np.float64
    WQ, WK, WO, Winv = (np.asarray(m) for m in (WQ, WK, WO, Winv))
    U_b, V_b, W_b = (np.asarray(m) for m in (U_b, V_b, W_b))
    U_t, V_t, W_t, X_t = (np.asarray(m) for m in (U_t, V_t, W_t, X_t))
    WQt = WQ.astype(f8).T
    WKt = WK.astype(f8).T
    Winvt = Winv.astype(f8).T
    # P' columns: [b3 | b1 | b7 | b2 | a3 | a1 | a2s]
    P = np.concatenate([
        WKt @ W_t.astype(f8),                              # b3
        WKt @ W_b.astype(f8),                              # b1
        X_t.astype(f8),                                    # b7
        WKt @ (Winvt @ V_b.astype(f8)),                    # b2
        WQt @ V_t.astype(f8),                              # a3
        WQt @ V_b.astype(f8),                              # a1
        float(alpha_bi) * (WQt @ (Winvt @ W_b.astype(f8))),  # a2s
    ], axis=1)
    # A' columns: [alpha_tri*WO@U_t | WO@U_b]  (GT rows: g2 then g1)
    A = np.concatenate([
        float(alpha_tri) * (WO.astype(f8) @ U_t.astype(f8)),
        WO.astype(f8) @ U_b.astype(f8),
    ], axis=1)
    return P, A


def make_in_maps(x, P, A):
    AT = np.ascontiguousarray(A.T.astype(np.float16))
    P16 = P.astype(np.float16)
    P5 = np.ascontiguousarray(P16.reshape(ND, 128, PCOLS).swapaxes(0, 1))
    in_maps = []
    for core in range(8):
        b, h = core // 2, core % 2
        xTc = x[b, h * TH:(h + 1) * TH, :].T.astype(np.float16)  # [D, TH]
        x5 = np.ascontiguousarray(
            xTc.reshape(ND // 2, 2, 128, 2, HW).transpose(2, 0, 3, 1, 4))
        if h == 1:
            sxP = x[b, :TH, :].astype(np.float64).sum(axis=0) @ P
            carry = np.stack([sxP[0:128], sxP[128:256]], axis=1)
        else:
            carry = np.zeros((128, 2), np.float64)
        counts = np.arange(h * TH + 1, (h + 1) * TH + 1, dtype=np.float64)
        invcr = (1.0 / counts).astype(np.float16)[None, :]
        in_maps.append(dict(x5=x5, P5=P5, AT=AT,
                            invcr=np.ascontiguousarray(invcr),
                            carry=np.ascontiguousarray(
                                carry.astype(np.float32))))
    return in_maps


def kernel(x, WQ, WK, WO, Winv, U_b, V_b, W_b, bias_b,
           U_t, V_t, W_t, X_t, bias_t, alpha_bi, alpha_tri):
    x = np.asarray(x, dtype=np.float32)
    P, A = _fold_weights(WQ, WK, WO, Winv, U_b, V_b, W_b,
                         U_t, V_t, W_t, X_t, alpha_bi, alpha_tri)
    in_maps = make_in_maps(x, P, A)

    res = run_bass_kernel_spmd(_get_nc(), in_maps, core_ids=list(range(8)))

    out = np.empty((B, T, D), np.float32)
    for core in range(8):
        b, h = core // 2, core % 2
        r = res.results[core]["out5"]  # [128, 2, ND, HW]
        outT = r.transpose(2, 0, 1, 3).reshape(D, TH)
        out[b, h * TH:(h + 1) * TH, :] = outT.T.astype(np.float32)

    # constant bias term (zero for the given inputs, kept for fidelity)
    bias_out = ((1.0 + float(alpha_bi)) * np.asarray(bias_b, np.float64)
                + float(alpha_tri) * np.asarray(bias_t, np.float64)) \
        @ np.asarray(WO, np.float64).T
    if np.any(bias_out):
        out += bias_out.astype(np.float32)[None, None, :]
    return out


# revision 8
# speedup vs baseline: 1.0768x; 1.0768x over previous
"""CausalBiTrilinearBCNAttention Trainium2 kernel (layout-B rewrite).

Math: the network collapses to xp = x @ P (448 rank columns), causal
cumsums over 4 of the 7 rank groups, elementwise rank products, and a
final [T,128]@[128,D] projection (see P/A folding below).

This version keeps FEATURES on partitions and TOKENS on the free dim:

  xpT = P.T @ x.T        4 stationary groups of <=128 P-columns,
                         tokens stream as the moving operand
  cums = tensor_tensor_scan (DVE prefix-add along free dim, fp32 state)
  ew   = lane-aligned DVE products (P column order is chosen so every
         product pairs values living on the same partitions):
           P cols = [b3|b1 | b7|b2 | a3|a1 | a2s]
           C0 = scan(G3) = [c3@lo | c1@hi]   C0p = C0 * invc
           C1 = scan(G4) = [c7@lo | c2@hi]   C1p = C1 * invc
           g2 = a3 * (c3'*c7')  @lo          -> GT[0:64]
           g1 = a1*c1' + a2s*c2'  @hi        -> GT[64:128]
  out  = A'.T @ GT       A' = [alpha_tri*WO@U_t | WO@U_b]

so there are no PE transposes, no PE cumsum/carry chain, and the PE
stream is 64 xpT MMs + 16 final MMs, all N=512, back to back (HAM
stays warm; a dummy-MM burst warms it during the DMA lead-in).

Sharding: 8 cores = 4 batches x 2 T-halves. The T/2 carry for the
second half is folded by the host into the scan initial values
(carry = sum_t x[b,:T/2] @ P, fp32) and chained across the two
512-token device halves via the scan output's last column.
"""

import numpy as np

import concourse.bass as bass
import concourse.tile as tile
from concourse import bacc, mybir
from concourse.bass_utils import run_bass_kernel_spmd

B, T, D, R = 4, 2048, 1024, 64
TH = T // 2          # tokens per core
ND = D // 128        # 8 contraction chunks
PCOLS = 448
HW = 512             # tokens per device half
F32 = mybir.dt.float32
F16 = mybir.dt.float16

# P' column groups (each <=128 wide -> one stationary tile):
#   G3  = cols   0:128 = [b3|b1]   (cumsummed)
#   G4  = cols 128:256 = [b7|b2]   (cumsummed)
#   A1  = cols 256:384 = [a3|a1]
#   A2s = cols 384:448 = [a2s]     (M=64, written to partitions 64:128)
G3_, G4_, A1_, A2s_ = (0, 128), (128, 256), (256, 384), (384, 448)

ADD = mybir.AluOpType.add
BYP = mybir.AluOpType.bypass
MUL = mybir.AluOpType.mult


def build_nc():
    nc = bacc.Bacc(None, target_bir_lowering=False)

    # x6: [128, half, chunk*tok] -- 2KB contiguous per partition per pair
    x6 = nc.dram_tensor("x6", [128, 2, ND * HW], F16, kind="ExternalInput")
    P5 = nc.dram_tensor("P5", [128, ND, PCOLS], F16, kind="ExternalInput")
    AT = nc.dram_tensor("AT", [128, D], F16, kind="ExternalInput")
    invcr = nc.dram_tensor("invcr", [1, TH], F16, kind="ExternalInput")
    carry = nc.dram_tensor("carry", [128, 2], F32, kind="ExternalInput")
    # out6: [128, half, chunk*tok] -- 2KB contiguous per pair store
    out6 = nc.dram_tensor("out6", [128, 2, ND * HW], F16,
                          kind="ExternalOutput")

    with tile.TileContext(nc) as tc:
        with tc.tile_pool(name="consts", bufs=1) as consts, \
             tc.tile_pool(name="big", bufs=1) as big, \
             tc.tile_pool(name="outp", bufs=4) as outp, \
             tc.tile_pool(name="ps", bufs=1, space="PSUM") as ps:

            # ---- HAM warmup + on-device invc broadcast ----
            warm_sb = consts.tile([128, 128], F16)
            nc.gpsimd.memset(warm_sb, 0.0)
            ones1 = consts.tile([1, 128], F16)
            nc.gpsimd.memset(ones1, 1.0)
            warm_ps = ps.tile([128, 512], F32, tag="out", bufs=3)
            for _ in range(10):
                nc.tensor.matmul(warm_ps[:, 0:128], warm_sb, warm_sb,
                                 start=True, stop=True)

            xh_sb = big.tile([128, 2, ND, HW], F16)
            P_sb = consts.tile([128, ND, PCOLS], F16)
            AT_sb = consts.tile([128, D], F16)
            invcr_sb = consts.tile([1, TH], F16)
            invcT_sb = consts.tile([128, TH], F16)
            carry_sb = consts.tile([128, 2], F32)

            C0raw = big.tile([128, 2, HW], F16)
            C1raw = big.tile([128, 2, HW], F16)
            C0p = big.tile([128, 2, HW], F16)
            C1p = big.tile([128, 2, HW], F16)
            A1sb = big.tile([128, 2, HW], F16)
            A2sb = big.tile([128, 2, HW], F16)
            t2sb = big.tile([64, 2, HW], F16)
            m1sb = big.tile([128, 2, HW], F16)
            m2sb = big.tile([128, 2, HW], F16)
            GT = big.tile([128, 2, HW], F16)

            # ---- input DMAs (per-partition-contiguous units) ----
            def ldx(q, j, h):
                q.dma_start(out=xh_sb[:, h, 2 * j:2 * j + 2, :],
                            in_=x6[:, h, 2 * j * HW:(2 * j + 2) * HW])

            ldx(nc.sync, 0, 0)
            nc.sync.dma_start(out=P_sb[:, 2:4, :], in_=P5[:, 2:4, :])
            ldx(nc.sync, 2, 0)
            ldx(nc.sync, 1, 1)
            ldx(nc.sync, 3, 1)
            nc.sync.dma_start(out=carry_sb, in_=carry[:, :])

            nc.scalar.dma_start(out=P_sb[:, 0:2, :], in_=P5[:, 0:2, :])
            ldx(nc.scalar, 1, 0)
            nc.scalar.dma_start(out=P_sb[:, 4:6, :], in_=P5[:, 4:6, :])
            ldx(nc.scalar, 3, 0)
            nc.scalar.dma_start(out=P_sb[:, 6:8, :], in_=P5[:, 6:8, :])
            ldx(nc.scalar, 0, 1)
            nc.scalar.dma_start(out=invcr_sb, in_=invcr[:, :])
            ldx(nc.scalar, 2, 1)
            nc.scalar.dma_start(out=AT_sb, in_=AT[:, :])

            def emit_invb():
                # invc broadcast rows: PE K=1 outer product, ScalarE drain
                for h in range(2):
                    ivp = ps.tile([128, HW], F32, tag="invb", bufs=1,
                                  name=f"ivp{h}")
                    nc.tensor.matmul(ivp, ones1,
                                     invcr_sb[0:1, h * HW:(h + 1) * HW],
                                     start=True, stop=True)
                    nc.scalar.copy(invcT_sb[:, h * HW:(h + 1) * HW], ivp)

            def xh(h, dk):
                return xh_sb[:, h, dk, :]

            def mm(gp, cols, h, dk):
                lo, hi = cols
                out_ap = gp[64:128, :] if cols is A2s_ else gp
                nc.tensor.matmul(out_ap, P_sb[:, dk, lo:hi], xh(h, dk),
                                 start=(dk == 0), stop=(dk == ND - 1))

            # ============ half 0: dk-pair-blocked sweep (DMA-paced) ========
            g3p = ps.tile([128, HW], F32, tag="g3", bufs=1)
            g4p = ps.tile([128, HW], F32, tag="g4", bufs=1)
            a1p = ps.tile([128, HW], F32, tag="a1", bufs=1)
            a2p = ps.tile([128, HW], F32, tag="a2s", bufs=1)
            for j in range(ND // 2):
                if j == 3:
                    emit_invb()
                for gp, cols in ((g3p, G3_), (g4p, G4_), (a1p, A1_),
                                 (a2p, A2s_)):
                    mm(gp, cols, 0, 2 * j)
                    mm(gp, cols, 0, 2 * j + 1)

            # h0 scans (raw cums; data1 is an ignored resident tile)
            dum = xh_sb[:, 0, 0, :]
            nc.vector.tensor_tensor_scan(C0raw[:, 0, :], g3p, dum,
                                         carry_sb[:, 0:1], ADD, BYP)
            nc.vector.tensor_tensor_scan(C1raw[:, 0, :], g4p, dum,
                                         carry_sb[:, 1:2], ADD, BYP)
            nc.vector.tensor_mul(C0p[:, 0, :], C0raw[:, 0, :],
                                 invcT_sb[:, 0:HW])
            nc.vector.tensor_mul(C1p[:, 0, :], C1raw[:, 0, :],
                                 invcT_sb[:, 0:HW])
            nc.scalar.copy(A1sb[:, 0, :], a1p)
            nc.scalar.copy(A2sb[64:128, 0, :], a2p[64:128, :])
            nc.vector.tensor_mul(m1sb[64:128, 0, :], A1sb[64:128, 0, :],
                                 C0p[64:128, 0, :])
            nc.vector.tensor_mul(m2sb[64:128, 0, :], A2sb[64:128, 0, :],
                                 C1p[64:128, 0, :])
            nc.vector.tensor_add(GT[64:128, 0, :], m1sb[64:128, 0, :],
                                 m2sb[64:128, 0, :])
            nc.gpsimd.tensor_mul(t2sb[:, 0, :], C0p[0:64, 0, :],
                                 C1p[0:64, 0, :])
            nc.vector.tensor_mul(GT[0:64, 0, :], A1sb[0:64, 0, :],
                                 t2sb[:, 0, :])

            # ============ half 1 =========
            g3p1 = ps.tile([128, HW], F32, tag="g3", bufs=1)
            for dk in range(ND):
                mm(g3p1, G3_, 1, dk)
            g4p1 = ps.tile([128, HW], F32, tag="g4", bufs=1)
            for dk in range(ND):
                mm(g4p1, G4_, 1, dk)

            nc.vector.tensor_tensor_scan(C0raw[:, 1, :], g3p1, dum,
                                         C0raw[:, 0, HW - 1:HW], ADD, BYP)
            nc.vector.tensor_tensor_scan(C1raw[:, 1, :], g4p1, dum,
                                         C1raw[:, 0, HW - 1:HW], ADD, BYP)
            nc.vector.tensor_mul(C0p[:, 1, :], C0raw[:, 1, :],
                                 invcT_sb[:, HW:TH])
            nc.vector.tensor_mul(C1p[:, 1, :], C1raw[:, 1, :],
                                 invcT_sb[:, HW:TH])

            a1p1 = ps.tile([128, HW], F32, tag="a1", bufs=1)
            for dk in range(ND):
                mm(a1p1, A1_, 1, dk)
            nc.scalar.copy(A1sb[:, 1, :], a1p1)

            def emit_final(h, vdks):
                for dk in range(ND):
                    o_ps = ps.tile([128, 512], F32, tag="out", bufs=3,
                                   name=f"ops{h}_{dk}")
                    nc.tensor.matmul(o_ps, AT_sb[:, dk * 128:(dk + 1) * 128],
                                     GT[:, h, :], start=True, stop=True)
                    if dk % 2 == 0:
                        osb = outp.tile([128, 2, HW], F16,
                                        name=f"osb{h}_{dk}")
                    if dk in vdks:
                        nc.vector.tensor_copy(osb[:, dk % 2, :], o_ps)
                    else:
                        nc.scalar.copy(osb[:, dk % 2, :], o_ps)
                    if dk % 2 == 1:
                        q = nc.sync if (dk // 2) % 2 == 0 else nc.scalar
                        q.dma_start(out=out6[:, h, (dk - 1) * HW:(dk + 1) * HW],
                                    in_=osb)

            nc.vector.tensor_mul(m1sb[64:128, 1, :], A1sb[64:128, 1, :],
                                 C0p[64:128, 1, :])
            nc.gpsimd.tensor_mul(t2sb[:, 1, :], C0p[0:64, 1, :],
                                 C1p[0:64, 1, :])
            nc.vector.tensor_mul(GT[0:64, 1, :], A1sb[0:64, 1, :],
                                 t2sb[:, 1, :])

            emit_final(0, vdks=(0, 2, 4, 6))

            a2p1 = ps.tile([128, HW], F32, tag="a2s", bufs=1)
            for dk in range(ND):
                mm(a2p1, A2s_, 1, dk)
            nc.scalar.copy(A2sb[64:128, 1, :], a2p1[64:128, :])
            nc.vector.tensor_mul(m2sb[64:128, 1, :], A2sb[64:128, 1, :],
                                 C1p[64:128, 1, :])
            nc.vector.tensor_add(GT[64:128, 1, :], m1sb[64:128, 1, :],
                                 m2sb[64:128, 1, :])

            emit_final(1, vdks=(0, 2, 4, 6))

    nc.finalize()
    return nc


_NC = None


def _get_nc():
    global _NC
    if _NC is None:
        _NC = build_nc()
    return _NC


def _fold_weights(WQ, WK, WO, Winv, U_b, V_b, W_b, U_t, V_t, W_t, X_t,
                  alpha_bi, alpha_tri):
    f8 = np.float64
    WQ, WK, WO, Winv = (np.asarray(m) for m in (WQ, WK, WO, Winv))
    U_b, V_b, W_b = (np.asarray(m) for m in (U_b, V_b, W_b))
    U_t, V_t, W_t, X_t = (np.asarray(m) for m in (U_t, V_t, W_t, X_t))
    WQt = WQ.astype(f8).T
    WKt = WK.astype(f8).T
    Winvt = Winv.astype(f8).T
    # P' columns: [b3 | b1 | b7 | b2 | a3 | a1 | a2s]
    P = np.concatenate([
        WKt @ W_t.astype(f8),                              # b3
        WKt @ W_b.astype(f8),                              # b1
        X_t.astype(f8),                                    # b7
        WKt @ (Winvt @ V_b.astype(f8)),                    # b2
        WQt @ V_t.astype(f8),                              # a3
        WQt @ V_b.astype(f8),                              # a1
        float(alpha_bi) * (WQt @ (Winvt @ W_b.astype(f8))),  # a2s
    ], axis=1)
    # A' columns: [alpha_tri*WO@U_t | WO@U_b]  (GT rows: g2 then g1)
    A = np.concatenate([
        float(alpha_tri) * (WO.astype(f8) @ U_t.astype(f8)),
        WO.astype(f8) @ U_b.astype(f8),
    ], axis=1)
    return P, A


def make_in_maps(x, P, A):
    AT = np.ascontiguousarray(A.T.astype(np.float16))
    P16 = P.astype(np.float16)
    P5 = np.ascontiguousarray(P16.reshape(ND, 128, PCOLS).swapaxes(0, 1))
    in_maps = []
    for core in range(8):
        b, h = core // 2, core % 2
        xTc = x[b, h * TH:(h + 1) * TH, :].T.astype(np.float16)  # [D, TH]
        x6 = np.ascontiguousarray(
            xTc.reshape(ND, 128, 2, HW).transpose(1, 2, 0, 3)
            .reshape(128, 2, ND * HW))
        if h == 1:
            sxP = x[b, :TH, :].astype(np.float64).sum(axis=0) @ P
            carry = np.stack([sxP[0:128], sxP[128:256]], axis=1)
        else:
            carry = np.zeros((128, 2), np.float64)
        counts = np.arange(h * TH + 1, (h + 1) * TH + 1, dtype=np.float64)
        invcr = (1.0 / counts).astype(np.float16)[None, :]
        in_maps.append(dict(x6=x6, P5=P5, AT=AT,
                            invcr=np.ascontiguousarray(invcr),
                            carry=np.ascontiguousarray(
                                carry.astype(np.float32))))
    return in_maps


def kernel(x, WQ, WK, WO, Winv, U_b, V_b, W_b, bias_b,
           U_t, V_t, W_t, X_t, bias_t, alpha_bi, alpha_tri):
    x = np.asarray(x, dtype=np.float32)
    P, A = _fold_weights(WQ, WK, WO, Winv, U_b, V_b, W_b,
                         U_t, V_t, W_t, X_t, alpha_bi, alpha_tri)
    in_maps = make_in_maps(x, P, A)

    res = run_bass_kernel_spmd(_get_nc(), in_maps, core_ids=list(range(8)))

    out = np.empty((B, T, D), np.float32)
    for core in range(8):
        b, h = core // 2, core % 2
        r = res.results[core]["out6"].reshape(128, 2, ND, HW)
        outT = r.transpose(2, 0, 1, 3).reshape(D, TH)
        out[b, h * TH:(h + 1) * TH, :] = outT.T.astype(np.float32)

    # constant bias term (zero for the given inputs, kept for fidelity)
    bias_out = ((1.0 + float(alpha_bi)) * np.asarray(bias_b, np.float64)
                + float(alpha_tri) * np.asarray(bias_t, np.float64)) \
        @ np.asarray(WO, np.float64).T
    if np.any(bias_out):
        out += bias_out.astype(np.float32)[None, None, :]
    return out
